# revision 7
# baseline (speedup 1.0000x reference)
"""AttnVLAD layer on 8 Trainium2 NeuronCores.

Data-parallel over batch: b=32 samples -> 4 per core. Host ships x once
in d-major fp16 (for mm1) plus the n-major fp16 copy for 3 of 4 banks;
the 4th bank's n-major tiles are produced on-device by PE transposes
(matmul against identity) to cut HBM traffic by 1/8th of a copy. The
fp16 split of q = alpha * centers/||centers|| is host-side, and the
global L2 normalize is folded into the cluster weights. Per sample:
  scoreT[n,K] = qh^T xh             (fp16 matmuls, fp32 PSUM accum)
  prob = softmax over K (fp16)      (one exp per 1024-n bank)
  descT[K,d] = prob^T @ xT          (fp16 matmuls, fp32 PSUM accum)
  denomT[K,1] rides mm2's stationary (prob^T @ ones)
  epilogue in [K,D] layout: denom-normalize, subtract centersT,
  intra-L2, weighted by cw/||cw|| -> out[K,D] (host transposes back)
"""
import numpy as np

B, D, N, K = 32, 512, 4096, 64
NCORES = 8
SPC = B // NCORES          # samples per core
DCH = D // 128             # 4 d-chunks
NCH = N // 128             # 32 n-chunks
BPB = 8                    # n-chunks per bank
NB = 4                     # banks per sample
NQN = 128 * BPB            # 1024 n per bank
NG = SPC * NB              # 16 global banks per core
SHIP = 3                   # xT banks shipped per sample (bank 3 generated)
HSL = 2048                 # xh slab n-width (2 banks)
NHS = N * SPC // HSL       # 8 xh slabs per core

_COMPILED = {}


def _build():
    import concourse.bass as bass
    import concourse.bacc as bacc
    import concourse.tile as tile
    import concourse.mybir as mybir

    f32 = mybir.dt.float32
    f16 = mybir.dt.float16
    AF = mybir.ActivationFunctionType
    OP = mybir.AluOpType
    AX = mybir.AxisListType

    nc = bacc.Bacc("TRN2", target_bir_lowering=False, debug=False)
    xh_dram = nc.dram_tensor("xh", [SPC, D, N], f16, kind="ExternalInput")
    xT_dram = nc.dram_tensor("xT", [SPC, N, D], f16, kind="ExternalInput")
    qh_dram = nc.dram_tensor("qh", [D, K], f16, kind="ExternalInput")
    id_dram = nc.dram_tensor("ident", [128, 128], f16, kind="ExternalInput")
    cT_dram = nc.dram_tensor("cT", [K, D], f32, kind="ExternalInput")
    cw_dram = nc.dram_tensor("cw", [K, 1], f32, kind="ExternalInput")
    out_dram = nc.dram_tensor("out", [SPC, K, D], f32, kind="ExternalOutput")

    with tile.TileContext(nc) as tc:
        with (
            tc.tile_pool(name="const", bufs=1) as const,
            tc.tile_pool(name="xhp", bufs=4) as xhp,
            tc.tile_pool(name="xsp", bufs=2) as xsp,
            tc.tile_pool(name="xgp", bufs=2) as xgp,
            tc.tile_pool(name="probp", bufs=2) as probp,
            tc.tile_pool(name="s16p", bufs=2) as s16p,
            tc.tile_pool(name="e16p", bufs=2) as e16p,
            tc.tile_pool(name="smp", bufs=4) as smp,
            tc.tile_pool(name="epp", bufs=1) as epp,
            tc.tile_pool(name="ps_sc", bufs=3, space="PSUM") as ps_sc,
            tc.tile_pool(name="ps_d", bufs=2, space="PSUM") as ps_d,
            tc.tile_pool(name="ps_n", bufs=1, space="PSUM") as ps_n,
            tc.tile_pool(name="ps_tp", bufs=2, space="PSUM") as ps_tp,
        ):
            xh_slabs = {}
            xT_ship = {}   # per-sample shipped banks 0..SHIP-1
            xT_gen = {}    # per-bank generated tiles (bank 3)

            def load_xh(h):
                t = xhp.tile([128, DCH, HSL], f16, tag="xh", name=f"xh{h}")
                s = h >> 1
                off = (h & 1) * HSL
                nc.sync.dma_start(
                    t[:], xh_dram[s, :, off:off + HSL]
                    .rearrange("(c p) n -> p c n", p=128))
                xh_slabs[h] = t

            def load_xT(s):
                t = xsp.tile([128, SHIP * BPB, DCH, 128], f16, tag="xs",
                             name=f"xs{s}")
                nc.sync.dma_start(
                    t[:], xT_dram[s, 0:SHIP * NQN, :]
                    .rearrange("(j p) (c f) -> p j c f", p=128, f=128))
                xT_ship[s] = t

            # first loads: qh (tiny, mm1 needs it), xh slab 0, xT sample 0
            qh_sb = const.tile([128, DCH, K], f16, tag="qh_sb")
            nc.sync.dma_start(
                qh_sb[:], qh_dram[:].rearrange("(c p) k -> p c k", p=128))
            load_xh(0)
            load_xT(0)
            load_xh(1)
            id_sb = const.tile([128, 128], f16, tag="id_sb")
            nc.sync.dma_start(id_sb[:], id_dram[:])
            ones16 = const.tile([128, 1], f16, tag="ones16")
            nc.gpsimd.memset(ones16[:], 1.0)
            cT_sb = const.tile([K, D], f32, tag="cT_sb")
            nc.sync.dma_start(cT_sb[:], cT_dram[:])
            cw_sb = const.tile([K, 1], f32, tag="cw_sb")
            nc.sync.dma_start(cw_sb[:], cw_dram[:])

            probs = {}
            descT = {}
            denomT = {}
            pending = []

            def mm1_bank(g):
                xh = xh_slabs[g >> 1]
                off = (g & 1) * NQN
                bank = ps_sc.tile([128, BPB, K], f32, tag="scoreT",
                                  name=f"scb{g}")
                first = True
                for dc in range(DCH):
                    for c in range(BPB):
                        last = (dc == DCH - 1 and c == BPB - 1)
                        nc.tensor.matmul(
                            bank[:, c, :],
                            xh[:, dc, off + c * 128:off + (c + 1) * 128],
                            qh_sb[:, dc, :],
                            start=first, stop=last,
                            skip_group_check=(not first))
                        first = False
                return bank

            def softmax_bank(g, bank):
                s, b = g >> 2, g & 3
                negmax = smp.tile([128, BPB], f32, tag="negmax")
                nc.vector.reduce_max(negmax[:].unsqueeze(2), bank[:],
                                     axis=AX.X, negate=True)
                s16 = s16p.tile([128, BPB, K], f16, tag="s16", name=f"s{g}")
                nc.vector.tensor_add(
                    s16[:], bank[:],
                    negmax[:].unsqueeze(2).broadcast_to([128, BPB, K]))
                e16 = e16p.tile([128, BPB, K], f16, tag="e16", name=f"e{g}")
                nc.scalar.activation(e16[:], s16[:], AF.Exp)
                rs = smp.tile([128, BPB], f32, tag="rs")
                nc.vector.reduce_sum(rs[:].unsqueeze(2), e16[:], axis=AX.X)
                rr = smp.tile([128, BPB], f32, tag="rr")
                nc.vector.reciprocal(rr[:], rs[:])
                rr16 = smp.tile([128, BPB], f16, tag="rr16")
                nc.vector.tensor_copy(rr16[:], rr[:])
                nc.vector.tensor_mul(
                    probs[s][:, b * BPB:(b + 1) * BPB, :], e16[:],
                    rr16[:].unsqueeze(2).broadcast_to([128, BPB, K]))

            def gen_xT(g):
                # PE transposes of bank 3's xh blocks -> n-major tiles
                xh = xh_slabs[g >> 1]
                off = (g & 1) * NQN
                t = xgp.tile([128, BPB, DCH, 128], f16, tag="xg",
                             name=f"xg{g}")
                for c in range(BPB):
                    tp = ps_tp.tile([128, DCH, 128], f32, tag="tp",
                                    name=f"tp{g}_{c}")
                    for dc in range(DCH):
                        nc.tensor.matmul(
                            tp[:, dc, :],
                            xh[:, dc, off + c * 128:off + (c + 1) * 128],
                            id_sb[:],
                            start=(dc == 0), stop=(dc == DCH - 1),
                            skip_group_check=(dc != 0))
                    nc.scalar.copy(t[:, c], tp[:])
                xT_gen[g] = t

            def mm2_bank(g):
                s, b = g >> 2, g & 3
                pr = probs[s]
                for c in range(BPB):
                    j = b * BPB + c
                    rhs = (xT_ship[s][:, j] if b < NB - 1
                           else xT_gen[g][:, c])
                    nc.tensor.matmul(descT[s][:], pr[:, j, :], rhs,
                                     start=(j == 0), stop=(j == NCH - 1),
                                     skip_group_check=(j != 0))
                    nc.tensor.matmul(denomT[s][:], pr[:, j, :], ones16[:],
                                     start=(j == 0), stop=(j == NCH - 1),
                                     skip_group_check=(j != 0))

            def make_epilogue(s, dT, dn):
                def run():
                    rdenom = epp.tile([K, 1], f32, tag="rdenom",
                                      name=f"rd{s}")
                    nc.vector.tensor_scalar_max(rdenom[:], dn[:], 1e-6)
                    nc.vector.reciprocal(rdenom[:], rdenom[:])
                    desc_c = epp.tile([K, D], f32, tag="desc_c",
                                      name=f"dcc{s}")
                    nc.vector.scalar_tensor_tensor(
                        desc_c[:], in0=dT[:], scalar=rdenom[:],
                        in1=cT_sb[:], op0=OP.mult, op1=OP.subtract)
                    sqe = epp.tile([K, D], f32, tag="sqe", name=f"sq{s}")
                    ss = epp.tile([K, 1], f32, tag="ss", name=f"ss{s}")
                    nc.vector.tensor_mul(sqe[:], desc_c[:], desc_c[:])
                    nc.vector.reduce_sum(ss[:], sqe[:], axis=AX.X)
                    intra = epp.tile([K, 1], f32, tag="intra", name=f"in{s}")
                    nc.scalar.activation(intra[:], ss[:], AF.Sqrt)
                    nc.vector.tensor_scalar_max(intra[:], intra[:], 1e-12)
                    rintra = epp.tile([K, 1], f32, tag="rintra",
                                      name=f"ri{s}")
                    nc.vector.reciprocal(rintra[:], intra[:])
                    sfin = epp.tile([K, 1], f32, tag="sfin", name=f"sf{s}")
                    nc.vector.tensor_mul(sfin[:], cw_sb[:], rintra[:])
                    outT = epp.tile([K, D], f32, tag="outT", name=f"oT{s}")
                    nc.vector.tensor_mul(outT[:], desc_c[:],
                                         sfin[:].broadcast_to([K, D]))
                    nc.sync.dma_start(out_dram[s], outT[:])
                return run

            for g in range(NG):
                s, b = g >> 2, g & 3
                # prefetch: xh two slabs ahead; xT one sample ahead
                if (g & 1) == 0 and (g >> 1) + 2 < NHS:
                    load_xh((g >> 1) + 2)
                if b == 1 and s + 1 < SPC:
                    load_xT(s + 1)
                if b == 0:
                    probs[s] = probp.tile([128, NCH, K], f16, tag="prob",
                                          name=f"pr{s}")
                bank = mm1_bank(g)
                softmax_bank(g, bank)
                if b == NB - 1:
                    gen_xT(g)
                if g >= 2:
                    g2 = g - 2
                    s2, b2 = g2 >> 2, g2 & 3
                    if b2 == 0:
                        descT[s2] = ps_d.tile([K, D], f32, tag="descT",
                                              name=f"dT{s2}")
                        denomT[s2] = ps_n.tile([K, 1], f32, tag="denomT",
                                               name=f"dn{s2}")
                    mm2_bank(g2)
                    if b2 == NB - 1:
                        pending.append(
                            make_epilogue(s2, descT[s2], denomT[s2]))
                if b == 2 and pending:
                    pending.pop(0)()

            for g2 in range(NG - 2, NG):
                s2, b2 = g2 >> 2, g2 & 3
                mm2_bank(g2)
                if b2 == NB - 1:
                    pending.append(make_epilogue(s2, descT[s2], denomT[s2]))
            for fn in pending:
                fn()
            pending.clear()

    nc.compile()
    return nc


def kernel(x, centers, alpha, cluster_weights):
    import concourse.bass_utils as bass_utils

    if "nc" not in _COMPILED:
        _COMPILED["nc"] = _build()
    nc = _COMPILED["nc"]

    x = np.asarray(x, dtype=np.float32)
    xh = np.ascontiguousarray(x.astype(np.float16))
    xT = np.ascontiguousarray(xh.transpose(0, 2, 1))

    c = np.asarray(centers, dtype=np.float64).reshape(D, K)
    a = float(np.asarray(alpha, dtype=np.float64))
    nrm = np.sqrt((c * c).sum(axis=0, keepdims=True))
    q = a * c / np.maximum(nrm, 1e-12)
    qh = q.astype(np.float16)
    cT = np.ascontiguousarray(c.T.astype(np.float32))
    cw = np.asarray(cluster_weights, dtype=np.float64).reshape(K, 1)
    # rows of desc are unit-L2 then scaled by cw, so the flattened norm
    # is ||cw||_2 exactly: fold the final normalize into cw.
    cw_eff = (cw / max(np.sqrt((cw * cw).sum()), 1e-12)).astype(np.float32)
    ident = np.eye(128, dtype=np.float16)

    in_maps = []
    for core in range(NCORES):
        in_maps.append({
            "xh": xh[core * SPC:(core + 1) * SPC],
            "xT": xT[core * SPC:(core + 1) * SPC],
            "qh": qh,
            "ident": ident,
            "cT": cT,
            "cw": cw_eff,
        })
    res = bass_utils.run_bass_kernel_spmd(nc, in_maps,
                                          core_ids=list(range(NCORES)))
    out = np.concatenate([res.results[i]["out"] for i in range(NCORES)],
                         axis=0)                       # [B, K, D]
    return np.ascontiguousarray(
        out.transpose(0, 2, 1).reshape(B, D * K)).astype(np.float32)


# revision 13
# speedup vs baseline: 1.1568x; 1.1568x over previous
"""AttnVLAD layer on 8 Trainium2 NeuronCores.

Data-parallel over batch: b=32 samples -> 4 per core. Host ships x once
in d-major fp16 (for mm1) plus the n-major fp16 copy for 3 of 4 banks;
the 4th bank's n-major tiles are produced on-device by PE transposes
(matmul against identity) to cut HBM traffic by 1/8th of a copy. The
fp16 split of q = alpha * centers/||centers|| is host-side, and the
global L2 normalize is folded into the cluster weights. Per sample:
  scoreT[n,K] = qh^T xh             (fp16 matmuls, fp32 PSUM accum)
  prob = softmax over K (fp16)      (one exp per 1024-n bank)
  descT[K,d] = prob^T @ xT          (fp16 matmuls, fp32 PSUM accum)
  denomT[K,1] rides mm2's stationary (prob^T @ ones)
  epilogue in [K,D] layout: denom-normalize, subtract centersT,
  intra-L2, weighted by cw/||cw|| -> out[K,D] (host transposes back)
"""
import numpy as np

B, D, N, K = 32, 512, 4096, 64
NCORES = 8
SPC = B // NCORES          # samples per core
DCH = D // 128             # 4 d-chunks
NCH = N // 128             # 32 n-chunks
BPB = 8                    # n-chunks per bank
NB = 4                     # banks per sample
NQN = 128 * BPB            # 1024 n per bank
NG = SPC * NB              # 16 global banks per core
SHIP = 3                   # xT banks shipped per sample (bank 3 generated)
HSL = 2048                 # xh slab n-width (2 banks)
NHS = N * SPC // HSL       # 8 xh slabs per core

_COMPILED = {}


def _build():
    import concourse.bass as bass
    import concourse.bacc as bacc
    import concourse.tile as tile
    import concourse.mybir as mybir

    f32 = mybir.dt.float32
    f16 = mybir.dt.float16
    AF = mybir.ActivationFunctionType
    OP = mybir.AluOpType
    AX = mybir.AxisListType

    nc = bacc.Bacc("TRN2", target_bir_lowering=False, debug=False)
    xh_dram = nc.dram_tensor("xh", [SPC, D, N], f16, kind="ExternalInput")
    xT_dram = nc.dram_tensor("xT", [SPC, N, D], f16, kind="ExternalInput")
    qh_dram = nc.dram_tensor("qh", [D, K], f16, kind="ExternalInput")
    id_dram = nc.dram_tensor("ident", [128, 128], f16, kind="ExternalInput")
    cT_dram = nc.dram_tensor("cT", [K, D], f32, kind="ExternalInput")
    cw_dram = nc.dram_tensor("cw", [K, 1], f32, kind="ExternalInput")
    out_dram = nc.dram_tensor("out", [SPC, K, D], f32, kind="ExternalOutput")

    with tile.TileContext(nc) as tc:
        with (
            tc.tile_pool(name="const", bufs=1) as const,
            tc.tile_pool(name="xhp", bufs=4) as xhp,
            tc.tile_pool(name="xsp", bufs=2) as xsp,
            tc.tile_pool(name="xgp", bufs=2) as xgp,
            tc.tile_pool(name="probp", bufs=2) as probp,
            tc.tile_pool(name="s16p", bufs=3) as s16p,
            tc.tile_pool(name="e16p", bufs=3) as e16p,
            tc.tile_pool(name="smp", bufs=8) as smp,
            tc.tile_pool(name="epp", bufs=1) as epp,
            tc.tile_pool(name="ps_sc", bufs=3, space="PSUM") as ps_sc,
            tc.tile_pool(name="ps_d", bufs=2, space="PSUM") as ps_d,
            tc.tile_pool(name="ps_n", bufs=1, space="PSUM") as ps_n,
            tc.tile_pool(name="ps_tp", bufs=2, space="PSUM") as ps_tp,
        ):
            xh_slabs = {}
            xT_ship = {}   # per-sample shipped banks 0..SHIP-1
            xT_gen = {}    # per-bank generated tiles (bank 3)

            def load_xh(h, split=False):
                t = xhp.tile([128, DCH, HSL], f16, tag="xh", name=f"xh{h}")
                s = h >> 1
                off = (h & 1) * HSL
                if split:
                    for q in range(2):
                        o = q * (HSL // 2)
                        nc.sync.dma_start(
                            t[:, :, o:o + HSL // 2],
                            xh_dram[s, :, off + o:off + o + HSL // 2]
                            .rearrange("(c p) n -> p c n", p=128))
                else:
                    nc.sync.dma_start(
                        t[:], xh_dram[s, :, off:off + HSL]
                        .rearrange("(c p) n -> p c n", p=128))
                xh_slabs[h] = t

            def load_xT(s, b):
                if b == 0:
                    xT_ship[s] = xsp.tile([128, SHIP * BPB, DCH, 128], f16,
                                          tag="xs", name=f"xs{s}")
                t = xT_ship[s]
                nc.sync.dma_start(
                    t[:, b * BPB:(b + 1) * BPB],
                    xT_dram[s, b * NQN:(b + 1) * NQN, :]
                    .rearrange("(j p) (c f) -> p j c f", p=128, f=128))

            # head loads: qh (tiny, mm1 needs it), then x pieces in the
            # order compute consumes them
            qh_sb = const.tile([128, DCH, K], f16, tag="qh_sb")
            nc.sync.dma_start(
                qh_sb[:], qh_dram[:].rearrange("(c p) k -> p c k", p=128))
            load_xh(0, split=True)
            load_xT(0, 0)
            load_xh(1)
            load_xT(0, 1)
            id_sb = const.tile([128, 128], f16, tag="id_sb")
            nc.sync.dma_start(id_sb[:], id_dram[:])
            ones16 = const.tile([128, 1], f16, tag="ones16")
            nc.gpsimd.memset(ones16[:], 1.0)
            load_xT(0, 2)
            cT_sb = const.tile([K, D], f32, tag="cT_sb")
            nc.sync.dma_start(cT_sb[:], cT_dram[:])
            cw_sb = const.tile([K, 1], f32, tag="cw_sb")
            nc.sync.dma_start(cw_sb[:], cw_dram[:])

            probs = {}
            descT = {}
            denomT = {}
            pending = []

            def mm1_bank(g):
                xh = xh_slabs[g >> 1]
                off = (g & 1) * NQN
                bank = ps_sc.tile([128, BPB, K], f32, tag="scoreT",
                                  name=f"scb{g}")
                first = True
                for dc in range(DCH):
                    for c in range(BPB):
                        last = (dc == DCH - 1 and c == BPB - 1)
                        nc.tensor.matmul(
                            bank[:, c, :],
                            xh[:, dc, off + c * 128:off + (c + 1) * 128],
                            qh_sb[:, dc, :],
                            start=first, stop=last,
                            skip_group_check=(not first))
                        first = False
                return bank

            def softmax_bank(g, bank):
                s, b = g >> 2, g & 3
                negmax = smp.tile([128, BPB], f32, tag="negmax")
                nc.vector.reduce_max(negmax[:].unsqueeze(2), bank[:],
                                     axis=AX.X, negate=True)
                s16 = s16p.tile([128, BPB, K], f16, tag="s16", name=f"s{g}")
                nc.vector.tensor_add(
                    s16[:], bank[:],
                    negmax[:].unsqueeze(2).broadcast_to([128, BPB, K]))
                e16 = e16p.tile([128, BPB, K], f16, tag="e16", name=f"e{g}")
                nc.scalar.activation(e16[:], s16[:], AF.Exp)
                rs = smp.tile([128, BPB], f32, tag="rs")
                nc.vector.reduce_sum(rs[:].unsqueeze(2), e16[:], axis=AX.X)
                rr = smp.tile([128, BPB], f32, tag="rr")
                nc.vector.reciprocal(rr[:], rs[:])
                rr16 = smp.tile([128, BPB], f16, tag="rr16")
                nc.vector.tensor_copy(rr16[:], rr[:])
                nc.gpsimd.tensor_mul(
                    probs[s][:, b * BPB:(b + 1) * BPB, :], e16[:],
                    rr16[:].unsqueeze(2).broadcast_to([128, BPB, K]))

            def gen_xT(g):
                # PE transposes of bank 3's xh blocks -> n-major tiles
                xh = xh_slabs[g >> 1]
                off = (g & 1) * NQN
                t = xgp.tile([128, BPB, DCH, 128], f16, tag="xg",
                             name=f"xg{g}")
                for c in range(BPB):
                    tp = ps_tp.tile([128, DCH, 128], f32, tag="tp",
                                    name=f"tp{g}_{c}")
                    for dc in range(DCH):
                        nc.tensor.matmul(
                            tp[:, dc, :],
                            xh[:, dc, off + c * 128:off + (c + 1) * 128],
                            id_sb[:],
                            start=(dc == 0), stop=(dc == DCH - 1),
                            skip_group_check=(dc != 0))
                    nc.scalar.copy(t[:, c], tp[:])
                xT_gen[g] = t

            def mm2_bank(g):
                s, b = g >> 2, g & 3
                pr = probs[s]
                for c in range(BPB):
                    j = b * BPB + c
                    rhs = (xT_ship[s][:, j] if b < NB - 1
                           else xT_gen[g][:, c])
                    nc.tensor.matmul(descT[s][:], pr[:, j, :], rhs,
                                     start=(j == 0), stop=(j == NCH - 1),
                                     skip_group_check=(j != 0))
                    nc.tensor.matmul(denomT[s][:], pr[:, j, :], ones16[:],
                                     start=(j == 0), stop=(j == NCH - 1),
                                     skip_group_check=(j != 0))

            def make_epilogue(s, dT, dn):
                def run():
                    rdenom = epp.tile([K, 1], f32, tag="rdenom",
                                      name=f"rd{s}")
                    nc.vector.tensor_scalar_max(rdenom[:], dn[:], 1e-6)
                    nc.vector.reciprocal(rdenom[:], rdenom[:])
                    desc_c = epp.tile([K, D], f32, tag="desc_c",
                                      name=f"dcc{s}")
                    nc.vector.scalar_tensor_tensor(
                        desc_c[:], in0=dT[:], scalar=rdenom[:],
                        in1=cT_sb[:], op0=OP.mult, op1=OP.subtract)
                    sqe = epp.tile([K, D], f32, tag="sqe", name=f"sq{s}")
                    ss = epp.tile([K, 1], f32, tag="ss", name=f"ss{s}")
                    nc.scalar.square(sqe[:], desc_c[:])
                    nc.vector.reduce_sum(ss[:], sqe[:], axis=AX.X)
                    intra = epp.tile([K, 1], f32, tag="intra", name=f"in{s}")
                    nc.scalar.activation(intra[:], ss[:], AF.Sqrt)
                    nc.vector.tensor_scalar_max(intra[:], intra[:], 1e-12)
                    rintra = epp.tile([K, 1], f32, tag="rintra",
                                      name=f"ri{s}")
                    nc.vector.reciprocal(rintra[:], intra[:])
                    sfin = epp.tile([K, 1], f32, tag="sfin", name=f"sf{s}")
                    nc.vector.tensor_mul(sfin[:], cw_sb[:], rintra[:])
                    outT = epp.tile([K, D], f32, tag="outT", name=f"oT{s}")
                    nc.gpsimd.tensor_mul(outT[:], desc_c[:],
                                         sfin[:].broadcast_to([K, D]))
                    nc.sync.dma_start(out_dram[s], outT[:])
                return run

            for g in range(NG):
                s, b = g >> 2, g & 3
                # prefetch: xh two slabs ahead; xT one sample ahead
                if (g & 1) == 0 and (g >> 1) + 2 < NHS:
                    load_xh((g >> 1) + 2)
                if b >= 1 and s + 1 < SPC:
                    load_xT(s + 1, b - 1)
                if b == 0:
                    probs[s] = probp.tile([128, NCH, K], f16, tag="prob",
                                          name=f"pr{s}")
                bank = mm1_bank(g)
                softmax_bank(g, bank)
                if b == NB - 1:
                    gen_xT(g)
                if g >= 2:
                    g2 = g - 2
                    s2, b2 = g2 >> 2, g2 & 3
                    if b2 == 0:
                        descT[s2] = ps_d.tile([K, D], f32, tag="descT",
                                              name=f"dT{s2}")
                        denomT[s2] = ps_n.tile([K, 1], f32, tag="denomT",
                                               name=f"dn{s2}")
                    mm2_bank(g2)
                    if b2 == NB - 1:
                        pending.append(
                            make_epilogue(s2, descT[s2], denomT[s2]))
                if b == 2 and pending:
                    pending.pop(0)()

            for g2 in range(NG - 2, NG):
                s2, b2 = g2 >> 2, g2 & 3
                mm2_bank(g2)
                if b2 == NB - 1:
                    pending.append(make_epilogue(s2, descT[s2], denomT[s2]))
            for fn in pending:
                fn()
            pending.clear()

    nc.compile()
    return nc


def kernel(x, centers, alpha, cluster_weights):
    import concourse.bass_utils as bass_utils

    if "nc" not in _COMPILED:
        _COMPILED["nc"] = _build()
    nc = _COMPILED["nc"]

    x = np.asarray(x, dtype=np.float32)
    xh = np.ascontiguousarray(x.astype(np.float16))
    xT = np.ascontiguousarray(xh.transpose(0, 2, 1))

    c = np.asarray(centers, dtype=np.float64).reshape(D, K)
    a = float(np.asarray(alpha, dtype=np.float64))
    nrm = np.sqrt((c * c).sum(axis=0, keepdims=True))
    q = a * c / np.maximum(nrm, 1e-12)
    qh = q.astype(np.float16)
    cT = np.ascontiguousarray(c.T.astype(np.float32))
    cw = np.asarray(cluster_weights, dtype=np.float64).reshape(K, 1)
    # rows of desc are unit-L2 then scaled by cw, so the flattened norm
    # is ||cw||_2 exactly: fold the final normalize into cw.
    cw_eff = (cw / max(np.sqrt((cw * cw).sum()), 1e-12)).astype(np.float32)
    ident = np.eye(128, dtype=np.float16)

    in_maps = []
    for core in range(NCORES):
        in_maps.append({
            "xh": xh[core * SPC:(core + 1) * SPC],
            "xT": xT[core * SPC:(core + 1) * SPC],
            "qh": qh,
            "ident": ident,
            "cT": cT,
            "cw": cw_eff,
        })
    res = bass_utils.run_bass_kernel_spmd(nc, in_maps,
                                          core_ids=list(range(NCORES)))
    out = np.concatenate([res.results[i]["out"] for i in range(NCORES)],
                         axis=0)                       # [B, K, D]
    return np.ascontiguousarray(
        out.transpose(0, 2, 1).reshape(B, D * K)).astype(np.float32)
